# revision 1
# baseline (speedup 1.0000x reference)
"""Trainium2 Bass kernel for the FHE BSGS conv-as-linear-transform problem.

Computes, for each batch row b of x[64, 65536]:
    out[b, s] = sum_{i=0}^{15} x[b, (s + 2^i * stride) % S] * diagonals[i, s]
    out = roll(out, S // (2 * reps))            (S = 65536)

Distribution: batch dim sharded across 8 NeuronCores (8 rows per core),
diagonals + stationary matrices replicated. No cross-core communication.

Per-core algorithm (free-minor layout: slot s = 512*p + f, p = partition):
  - A shift delta = 512*m + df decomposes into a free-dim offset df (read
    the x tile at column offset df against a 256-column halo) and a
    partition rotation by m (folded into the accumulation matmul).
  - VectorE computes the 16 elementwise products in fp16 (2x perf mode),
    fused over all 8 batches via broadcast (stride-0) diagonal operands,
    and fused over transform groups whose column offsets form an even
    arithmetic progression. Odd offsets read a duplicate x tile shifted
    by one slot (xh2) so the access stays 4-byte aligned (keeps 2x mode).
  - TensorE accumulates each product tile into a per-batch PSUM bank via a
    128x128 rotation-permutation stationary matmul; the partition rotation
    of each shift AND the final roll (multiple of 512 slots) are folded
    into these matrices, so PSUM holds the final rolled output directly.
  - Diagonals are pre-rotated along the partition axis on the host (pure
    relayout), so the product stage needs no on-chip data movement.
  - PSUM eviction split across VectorE + ScalarE, then straight DMA out.

All device-input arrays are prepared host-side in the exact SBUF layout so
every input DMA is a dense [128, N] partition-major copy.
"""

import os
import sys

for _p in ("/opt/trn_rl_repo",):
    if os.path.isdir(_p) and _p not in sys.path:
        sys.path.insert(0, _p)

import numpy as np

import concourse.bass as bass
import concourse.mybir as mybir
from concourse import bacc
from concourse.bass_utils import run_bass_kernel_spmd
from concourse.tile import TileContext

N_CORES = 8
BATCH = 64
SLOTS = 65536
NUM_T = 16
P = 128                 # SBUF partitions
F = SLOTS // P          # 512 slots per partition per batch
BPC = BATCH // N_CORES  # 8 batch rows per core
HALO = 256              # halo columns (covers df <= 256 for stride=1)
XPITCH = F + HALO       # 768
MAX_GROUP = 4           # max transforms fused into one DVE op


def _decomp(stride, reps):
    """Per-transform (m_i, df_i) shift decomposition + stationary rotations."""
    roll = (SLOTS // (2 * reps)) % SLOTS
    assert roll % F == 0, f"final roll {roll} not a multiple of {F}"
    mr = roll // F
    dec = []
    for i in range(NUM_T):
        delta = ((1 << i) * stride) % SLOTS
        dec.append((delta // F, delta % F))
    rots = [(m - mr) % P for (m, _) in dec]
    uniq = sorted(set(rots))
    sidx = {a: j for j, a in enumerate(uniq)}
    return dec, rots, uniq, sidx


def _plan_groups(dec):
    """Group transforms into fused DVE ops.

    Each group: (src, base, step, [i...]) where src is 'X' (offset df) or
    'X2' (offset df-1, for odd df), and the offsets of members form an
    arithmetic progression base + k*step with base and step even (step may
    be 0). Singletons always qualify.
    """
    items = []  # (src, off, i)
    for i, (_, df) in enumerate(dec):
        if df % 2 == 1:
            items.append(("X2", df - 1, i))
        else:
            items.append(("X", df, i))
    # group by src, sorted by offset; chain equal-step even progressions
    groups = []
    for src in ("X2", "X"):
        rows = sorted([it for it in items if it[0] == src], key=lambda t: t[1])
        k = 0
        while k < len(rows):
            base = rows[k][1]
            members = [rows[k][2]]
            step = None
            j = k + 1
            while j < len(rows) and len(members) < MAX_GROUP:
                d = rows[j][1] - rows[j - 1][1]
                if d % 2 != 0:
                    break
                if step is None:
                    step = d
                elif d != step:
                    break
                members.append(rows[j][2])
                j += 1
            groups.append((src, base, step or 0, members))
            k = j
    # execution order: big stride-0 X groups first (only need X + the first
    # dd columns, and their long matmul bursts warm the PE), X2 groups in
    # the middle (X2 loads behind X on the same queue), the rest last
    def order_key(g):
        src, base, step, mem = g
        if src == "X" and step == 0 and len(mem) > 1:
            return (0, base)
        if src == "X2":
            return (1, base)
        return (2, base)
    groups.sort(key=order_key)
    return groups


def build_module(stride=1, reps=1, debug=False, repeat=1, parts="all"):
    """Build + finalize the per-core Bass module (same program on all cores)."""
    dec, rots, uniq, sidx = _decomp(stride, reps)
    groups = _plan_groups(dec)
    ns = len(uniq)
    # dd column layout follows group order; stationary matrices are packed
    # in front of the diagonals in the same DRAM tensor ("ds") so one DMA
    # chain loads both.
    dd_order = [i for (_, _, _, mem) in groups for i in mem]
    dd_col = {i: k for k, i in enumerate(dd_order)}
    st_cols = ns * P                       # stat columns at front of ds
    n_first = st_cols + len(groups[0][3]) * F  # ds columns the 1st group needs

    f16 = mybir.dt.float16
    f32 = mybir.dt.float32

    nc = bacc.Bacc("TRN2", target_bir_lowering=False, debug=debug,
                   num_devices=N_CORES)
    xh = nc.dram_tensor("xh", [P, BPC * XPITCH], f16, kind="ExternalInput")
    xh2 = nc.dram_tensor("xh2", [P, BPC * XPITCH], f16, kind="ExternalInput")
    ds = nc.dram_tensor("ds", [P, st_cols + NUM_T * F], f16,
                        kind="ExternalInput")
    y = nc.dram_tensor("y", [BPC, SLOTS], f32, kind="ExternalOutput")

    use_x2 = any(src == "X2" for (src, _, _, _) in groups)

    with TileContext(nc) as tc:
        with (
            tc.tile_pool(name="xa", bufs=1) as xa_pool,
            tc.tile_pool(name="dda", bufs=1) as dd_pool,
            tc.tile_pool(name="prod", bufs=3) as prod_pool,
            tc.tile_pool(name="outs", bufs=1) as out_pool,
            tc.tile_pool(name="ps", bufs=1, space="PSUM") as ps_pool,
        ):
            DS = dd_pool.tile([P, st_cols + NUM_T * F], f16)
            X = xa_pool.tile([P, BPC * XPITCH], f16)
            # sync ring: Xa, X2, out-half-A;
            # scalar ring: DS(first), Xb, DS(rest), out-half-B.
            # First group needs X + DS[:n_first] only.
            xmid = (BPC // 2) * XPITCH
            nc.sync.dma_start(out=X[:, :xmid], in_=xh[:, :xmid])
            nc.scalar.dma_start(out=DS[:, :n_first], in_=ds[:, :n_first])
            nc.scalar.dma_start(out=X[:, xmid:], in_=xh[:, xmid:])
            nc.scalar.dma_start(out=DS[:, n_first:], in_=ds[:, n_first:])
            if use_x2:
                X2 = xa_pool.tile([P, BPC * XPITCH], f16, name="X2", tag="X2")
                nc.sync.dma_start(out=X2[:], in_=xh2[:, :])

            pstat = None
            if parts == "pe":
                # PE-only variant reads a pre-zeroed static product tile
                pstat = prod_pool.tile([P, MAX_GROUP * BPC * F], f16,
                                       name="pstat", tag="prod")
                nc.gpsimd.memset(pstat[:], 0.0)

            def body(_iv=None):
                psums = [
                    ps_pool.tile([P, F], f32, name=f"psum{b}", tag=f"ps{b}",
                                 bufs=1)
                    for b in range(BPC)
                ]
                for gi, (src, base, step, mem) in enumerate(groups):
                    ng = len(mem)
                    prod = pstat if parts == "pe" else prod_pool.tile(
                        [P, ng * BPC * F], f16, name="prod", tag="prod",
                        padded_shape=[P, MAX_GROUP * BPC * F])
                    xt = X2 if src == "X2" else X
                    in0 = bass.AP(
                        xt.tensor, xt.offset + base,
                        [list(xt.ap[0]), [step, ng], [XPITCH, BPC], [1, F]],
                    )
                    c0 = st_cols + dd_col[mem[0]] * F
                    dsl = DS[:, c0:c0 + ng * F]
                    in1 = bass.AP(
                        dsl.tensor, dsl.offset,
                        [list(dsl.ap[0]), [F, ng], [0, BPC], [1, F]],
                    )
                    out4 = prod[:].rearrange("p (g b f) -> p g b f",
                                             b=BPC, f=F)
                    if parts in ("all", "dve", "dvepe"):
                        nc.vector.tensor_mul(out4, in0, in1)
                    if parts not in ("all", "pe", "dvepe"):
                        continue
                    for idx, i in enumerate(mem):
                        lhsT = DS[:, sidx[rots[i]] * P:(sidx[rots[i]] + 1) * P]
                        for b in range(BPC):
                            nc.tensor.matmul(
                                psums[b][:], lhsT,
                                prod[:, (idx * BPC + b) * F:
                                        (idx * BPC + b + 1) * F],
                                start=(gi == 0 and idx == 0),
                                stop=(gi == len(groups) - 1 and
                                      idx == ng - 1),
                            )
                if parts == "empty":
                    tik = out_pool.tile([1, 1], f32, name="tik", tag="tik",
                                        bufs=1)
                    nc.gpsimd.memset(tik[:, :], 0.0)
                if parts not in ("all", "evict"):
                    return
                half = BPC // 2
                for h in range(2):
                    ot = out_pool.tile([P, half * F], f32, name=f"ot{h}",
                                       tag=f"ot{h}", bufs=2)
                    for k in range(half):
                        b = h * half + k
                        dst = ot[:, k * F:(k + 1) * F]
                        if h == 0:
                            nc.vector.tensor_copy(dst, psums[b][:])
                        else:
                            nc.scalar.copy(dst, psums[b][:])
                    if parts != "all":
                        continue
                    ydst = y[h * half:(h + 1) * half, :].rearrange(
                        "b (p f) -> p b f", f=F)
                    eng = nc.sync if h == 0 else nc.scalar
                    eng.dma_start(out=ydst, in_=ot[:].rearrange(
                        "p (b f) -> p b f", f=F))

            if repeat == 1:
                body()
            else:
                with tc.For_i(0, repeat, 1):
                    body()
    nc.finalize()
    return nc


def prep_inputs(x, diagonals, stride=1, reps=1):
    """Host-side shard + relayout. Returns in_maps for run_bass_kernel_spmd."""
    dec, rots, uniq, _ = _decomp(stride, reps)
    groups = _plan_groups(dec)
    ns = len(uniq)
    dd_order = [i for (_, _, _, mem) in groups for i in mem]

    x16 = np.ascontiguousarray(x, dtype=np.float16)
    # halo tiles in SBUF layout: xh[p, b*XPITCH + j] = x[b, (512p + j) % S]
    j = np.arange(XPITCH)
    idx = (np.arange(P)[:, None] * F + j[None, :]) % SLOTS
    xt = x16[:, idx]                       # [BATCH, P, XPITCH]
    xh = np.ascontiguousarray(
        np.stack([np.transpose(xt[c * BPC:(c + 1) * BPC], (1, 0, 2))
                  .reshape(P, BPC * XPITCH) for c in range(N_CORES)])
    )
    # xh2[b, p, j] = x[b, (512p + j + 1) % S]: shift xh left by one column,
    # refilling the last halo column instead of re-gathering everything.
    xt2 = np.empty_like(xt)
    xt2[:, :, :-1] = xt[:, :, 1:]
    xt2[:, :, -1] = x16[:, (np.arange(P) * F + XPITCH) % SLOTS]
    xh2 = np.ascontiguousarray(
        np.stack([np.transpose(xt2[c * BPC:(c + 1) * BPC], (1, 0, 2))
                  .reshape(P, BPC * XPITCH) for c in range(N_CORES)])
    )

    d16 = np.asarray(diagonals, dtype=np.float16).reshape(NUM_T, P, F)
    ddl = [np.roll(d16[i], dec[i][0], axis=0) for i in dd_order]
    dd = np.transpose(np.stack(ddl), (1, 0, 2)).reshape(P, NUM_T * F)

    st = np.zeros((ns, P, P), np.float16)
    cols = np.arange(P)
    for k, a in enumerate(uniq):
        st[k, (cols + a) % P, cols] = 1.0
    st = np.transpose(st, (1, 0, 2)).reshape(P, ns * P)

    ds = np.ascontiguousarray(np.concatenate([st, dd], axis=1))

    in_maps = []
    for c in range(N_CORES):
        in_maps.append({
            "xh": xh[c],
            "xh2": xh2[c],
            "ds": ds,
        })
    return in_maps


_MODULE_CACHE = {}


def kernel(**inputs):
    x = np.asarray(inputs["x"], dtype=np.float32)
    diagonals = np.asarray(inputs["diagonals"], dtype=np.float32)
    stride = int(np.asarray(inputs.get("stride", 1)))
    reps = int(np.asarray(inputs.get("reps", 1)))
    assert x.shape == (BATCH, SLOTS) and diagonals.shape == (NUM_T, SLOTS)
    # halo must cover the largest in-partition shift
    dec, _, _, _ = _decomp(stride, reps)
    assert max(df for _, df in dec) <= HALO, "halo too small for this stride"

    key = (stride, reps)
    if key not in _MODULE_CACHE:
        _MODULE_CACHE[key] = build_module(stride, reps)
    nc = _MODULE_CACHE[key]

    in_maps = prep_inputs(x, diagonals, stride, reps)
    res = run_bass_kernel_spmd(nc, in_maps, list(range(N_CORES)))
    out = np.concatenate(
        [np.asarray(res.results[c]["y"]) for c in range(N_CORES)], axis=0
    )
    return out.astype(np.float32)

